# revision 14
# baseline (speedup 1.0000x reference)
"""MLA attention (DeepSeek-style) Trainium2 Bass kernel.

Sharding: 8 cores = 2 batches x 4 head-groups (4 heads each). The shared
low-rank projections (q_a + rmsnorm, kv latent + rmsnorm, roped k_rot)
are token-sharded within each batch group: core g computes them for its
own 512-token chunk only, then 4-core DRAM AllGathers replicate the tiny
normalized latents. Attention + o-proj stay head-sharded (tensor
parallel); the host sums the 4 bf16 o-proj partials per batch.

Collective/compute overlap: the latent AllGather (A) is issued right
after the kv-latent matmuls and hides behind the q_a matmuls; the q_a
AllGather is split into two token-halves (B1/B2) that pipeline on the CC
stream. k_nope/V (need only A) overlap B1; attention for the first
token-half of every chunk (needs only B1) overlaps B2.

Layout convention on device: activations are feature-major "FM"
[feature on partitions, tokens on free dim]. Scores are [k, q] so that
softmax denominators / PV matmuls need no transposes anywhere.
"""

import sys
import numpy as np

sys.path.insert(0, "/opt/trn_rl_repo")

import ml_dtypes  # noqa: E402

import concourse.bass as bass  # noqa: E402
import concourse.bacc as bacc  # noqa: E402
import concourse.tile as tile  # noqa: E402
from concourse.tile_rust import add_dep_helper  # noqa: E402
from concourse import mybir  # noqa: E402
from concourse import bass_isa  # noqa: E402
from concourse.bass_utils import run_bass_kernel_spmd  # noqa: E402

F32 = mybir.dt.float32
F32R = mybir.dt.float32r
BF16 = mybir.dt.bfloat16
AF = mybir.ActivationFunctionType
ALU = mybir.AluOpType

# problem constants (hardcoded per contract)
B, S, HID = 2, 2048, 2048
H, D_NOPE, D_ROPE, D_V = 16, 128, 64, 128
D_QK = D_NOPE + D_ROPE
Q_RANK, KV_RANK = 1536, 512
EPS = 1e-6
SCALING = D_QK ** -0.5
NEG = -1.0e5  # causal mask additive constant (pre-scaling); exp -> 0

HPC = 4                      # heads per core
NCHUNK = 4                   # token chunks of 512
CH = S // NCHUNK             # 512
HC = CH // 2                 # 256 (attention token-half)
KT = S // 128                # 16 k tiles
QA_M = Q_RANK // 128         # 12
QB_M = (HPC * D_QK) // 128   # 6 (4 nope tiles + 2 rot tiles)
HID_K = HID // 128           # 16
KV_M = KV_RANK // 128        # 4
GT = QA_M + KV_M + 1         # 17 gather tiles: 4 latent + 1 krot + 12 qa
RG = [[0, 1, 2, 3], [4, 5, 6, 7]]


def build_nc():
    nc = bacc.Bacc(num_devices=8)

    # ---- I/O ----
    hT = nc.declare_dram_parameter("hT", [HID, CH], BF16, isOutput=False)
    w_qa = nc.declare_dram_parameter("w_qa", [HID, Q_RANK], BF16, isOutput=False)
    w_qb = nc.declare_dram_parameter("w_qb", [Q_RANK, HPC * D_QK], BF16, isOutput=False)
    w_kva = nc.declare_dram_parameter("w_kva", [HID, KV_RANK + 128], BF16, isOutput=False)
    w_kvb_n = nc.declare_dram_parameter("w_kvb_n", [KV_RANK, HPC * D_NOPE], BF16, isOutput=False)
    w_kvb_v = nc.declare_dram_parameter("w_kvb_v", [KV_RANK, HPC * D_V], BF16, isOutput=False)
    w_o = nc.declare_dram_parameter("w_o", [HPC * D_V, HID], BF16, isOutput=False)
    cos2 = nc.declare_dram_parameter("cos2", [128, S], F32, isOutput=False)
    sin2 = nc.declare_dram_parameter("sin2", [128, S], F32, isOutput=False)
    cosk = nc.declare_dram_parameter("cosk", [128, CH], F32, isOutput=False)
    sink = nc.declare_dram_parameter("sink", [128, CH], F32, isOutput=False)
    r2 = nc.declare_dram_parameter("r2", [128, 128], F32, isOutput=False)
    masks = nc.declare_dram_parameter("masks", [4, 128, CH], F32, isOutput=False)
    out = nc.declare_dram_parameter("out", [S, HID], BF16, isOutput=True)

    with tile.TileContext(nc) as tc:
        _emit(nc, tc, hT, w_qa, w_qb, w_kva, w_kvb_n, w_kvb_v, w_o,
              cos2, sin2, cosk, sink, r2, masks, out)
    nc.finalize()
    return nc


def _emit(nc, tc, hT, w_qa, w_qb, w_kva, w_kvb_n, w_kvb_v, w_o,
          cos2, sin2, cosk, sink, r2, masks, out):
    from contextlib import ExitStack

    (hT, w_qa, w_qb, w_kva, w_kvb_n, w_kvb_v, w_o, cos2, sin2, cosk, sink,
     r2, masks, out) = (
        x.ap() for x in (hT, w_qa, w_qb, w_kva, w_kvb_n, w_kvb_v, w_o,
                         cos2, sin2, cosk, sink, r2, masks, out))

    es = ExitStack()
    with es:
        # ------- tiny constants + long-lived activations -------
        tiny = es.enter_context(tc.tile_pool(name="tiny", bufs=1))
        ones_src = tiny.tile([128, 1], F32)
        nc.vector.memset(ones_src, 1.0)
        ones_col_bf = tiny.tile([128, 1], BF16)        # denom reducer lhsT
        nc.vector.memset(ones_col_bf, 1.0)
        ones_row_src = tiny.tile([1, 128], F32)
        nc.vector.memset(ones_row_src, 1.0)
        ones_row = tiny.tile([1, 128], F32R)            # fence lhsT
        nc.vector.tensor_copy(out=ones_row, in_=ones_row_src)
        eps_sb = tiny.tile([128, 1], F32)              # rmsnorm eps bias
        nc.vector.memset(eps_sb, EPS)
        r2_stage = tiny.tile([128, 128], F32)
        nc.sync.dma_start(out=r2_stage, in_=r2)
        r2_sb = tiny.tile([128, 128], F32R)
        nc.vector.tensor_copy(out=r2_sb, in_=r2_stage)  # DVE-produced (1-wait rule)

        psF = es.enter_context(tc.tile_pool(name="psF", bufs=1, space="PSUM"))
        fence_ps = psF.tile([1, 8], F32)

        # persistent activations consumed by attention
        qpass = es.enter_context(tc.tile_pool(name="qpass", bufs=1))
        qpass_sb = qpass.tile([128, HPC, S], BF16)     # qf nope, per head
        qrot_pool = es.enter_context(tc.tile_pool(name="qrot", bufs=1))
        qrot_sb = qrot_pool.tile([128, 2, S], BF16)    # qf rot, 2 heads per tile
        krot_pool = es.enter_context(tc.tile_pool(name="krot", bufs=1))
        krot_sb = krot_pool.tile([128, S], BF16)       # k rot (dup'd 64+64)
        kn_pool = es.enter_context(tc.tile_pool(name="kn", bufs=1))
        kn_sb = kn_pool.tile([128, HPC, S], BF16)
        v_pool = es.enter_context(tc.tile_pool(name="vtm", bufs=1))
        v_sb = v_pool.tile([128, KT, HPC * D_V], BF16)

        # DRAM bounce + gather buffers for the group AllGathers.
        dramp = es.enter_context(tc.tile_pool(name="dram", bufs=1, space="DRAM"))
        bounceA = dramp.tile([(KV_M + 1) * 128, CH], BF16)
        gathA = dramp.tile([NCHUNK * (KV_M + 1) * 128, CH], BF16)
        bounceB = [dramp.tile([QA_M * 128, HC], BF16, name=f"bounceB{i}")
                   for i in range(2)]
        gathB = [dramp.tile([NCHUNK * QA_M * 128, HC], BF16, name=f"gathB{i}")
                 for i in range(2)]

        last = {}  # most recent instruction handle per engine key

        def pe_observe(*deps):
            """Emit chained trivial PE matmuls, each sync-depending on one
            producer, so later PE matmuls don't accumulate multi-sem waits
            (fused-weight-load matmuls have few sync-wait slots in walrus).
            All write the same dedicated fence psum tile (same-engine WAW
            needs no semaphores). Returns the last absorber; phase-first
            matmuls must nosync-depend on it."""
            n = None
            for d in deps:
                if d is None:
                    continue
                prev = n
                n = nc.tensor.matmul(fence_ps[:, 0:8], ones_row[:, 0:1],
                                     ones_row[:, 0:8], start=True, stop=True,
                                     skip_group_check=True)
                add_dep_helper(n.ins, d.ins, True,
                               "phase-boundary PE observation")
                if prev is not None:
                    add_dep_helper(n.ins, prev.ins, False, "fence chain order")
            return n

        def rmsnorm_scale(sq_acc, sca, rank):
            """[128,CH] accumulated squares -> [128,CH] broadcast 1/rms."""
            ssq_bc = sca.tile([128, CH], F32, tag="ssqbc")
            nc.gpsimd.partition_all_reduce(ssq_bc, sq_acc, 128,
                                           bass_isa.ReduceOp.add)
            s_t = sca.tile([128, CH], F32, tag="srow")
            last["act"] = nc.scalar.activation(s_t, ssq_bc, AF.Sqrt,
                                               bias=eps_sb, scale=1.0 / rank)
            s_bc = sca.tile([128, CH], F32, tag="sbcs")
            last["dve"] = nc.vector.reciprocal(s_bc, s_t)
            return s_bc

        def rope(x_ps, cos_ap, sin_ap, out_ap, tmps, psX, width=CH):
            """RoPE a [128,width] psum tile (two 64-dim halves) -> out_ap."""
            xr = tmps.tile([128, width], F32R, tag="xr")
            nc.vector.tensor_copy(out=xr, in_=x_ps)
            rx_ps = psX.tile([128, width], F32, tag="scores")
            nc.tensor.matmul(rx_ps, r2_sb, xr)
            a_t = tmps.tile([128, width], F32, tag="ra")
            nc.vector.tensor_mul(a_t, xr, cos_ap)
            b_t = tmps.tile([128, width], F32, tag="rb")
            nc.vector.tensor_mul(b_t, rx_ps, sin_ap)
            return nc.vector.tensor_tensor(out_ap, a_t, b_t, ALU.add)

        # ============ PHASE L: local-chunk kv latent / k_rot / q_a ========
        pl = ExitStack()
        with pl:
            lconst = pl.enter_context(tc.tile_pool(name="lconst", bufs=1))
            ht_sb = lconst.tile([128, HID_K, CH], BF16)
            wkva_sb = lconst.tile([128, HID_K, KV_RANK + 128], BF16)
            # split the first loads so the first matmuls start sooner
            HK2 = HID_K // 2
            nc.sync.dma_start(
                out=ht_sb[:, 0:HK2, :],
                in_=hT[0:HK2 * 128, :].rearrange("(ko p) t -> p ko t", p=128))
            nc.sync.dma_start(
                out=wkva_sb[:, 0:HK2, :],
                in_=w_kva[0:HK2 * 128, :].rearrange("(ko p) m -> p ko m", p=128))
            nc.sync.dma_start(
                out=ht_sb[:, HK2:, :],
                in_=hT[HK2 * 128:, :].rearrange("(ko p) t -> p ko t", p=128))
            nc.sync.dma_start(
                out=wkva_sb[:, HK2:, :],
                in_=w_kva[HK2 * 128:, :].rearrange("(ko p) m -> p ko m", p=128))
            cosk_sb = lconst.tile([128, CH], F32)
            nc.sync.dma_start(out=cosk_sb, in_=cosk)
            sink_sb = lconst.tile([128, CH], F32)
            nc.sync.dma_start(out=sink_sb, in_=sink)
            wqa_pool = pl.enter_context(tc.tile_pool(name="wqa", bufs=2))

            qa_st = pl.enter_context(tc.tile_pool(name="qast", bufs=1))
            qa_sb = qa_st.tile([128, QA_M, CH], F32)
            gsrcp = pl.enter_context(tc.tile_pool(name="gsrc", bufs=1))
            gsrc = gsrcp.tile([128, GT, CH], BF16)

            tmps = pl.enter_context(tc.tile_pool(name="tmpsL", bufs=2))
            sca = pl.enter_context(tc.tile_pool(name="scaleL", bufs=2))
            psA = pl.enter_context(tc.tile_pool(name="psA", bufs=2, space="PSUM"))
            psX = pl.enter_context(tc.tile_pool(name="psX", bufs=1, space="PSUM"))
            psLat = pl.enter_context(tc.tile_pool(name="psLat", bufs=1, space="PSUM"))

            # ---- kv latent for own chunk (feeds the early AllGather) ----
            sq_acc2 = tmps.tile([128, CH], F32R, tag="sqacc2")
            lat_ps = []
            for m in range(KV_M):
                l_ps = psLat.tile([128, CH], F32, tag=f"lat{m}")
                lat_ps.append(l_ps)
                for k in range(HID_K):
                    last["pe"] = nc.tensor.matmul(
                        l_ps, wkva_sb[:, k, m * 128:(m + 1) * 128],
                        ht_sb[:, k, :],
                        start=(k == 0), stop=(k == HID_K - 1))
                if m == 0:
                    last["act"] = nc.scalar.activation(sq_acc2, l_ps, AF.Square)
                else:
                    sq = tmps.tile([128, CH], F32R, tag="sq")
                    last["act"] = nc.scalar.activation(sq, l_ps, AF.Square)
                    nc.gpsimd.tensor_tensor(sq_acc2, sq_acc2, sq, ALU.add)

            s_bc2 = rmsnorm_scale(sq_acc2, sca, KV_RANK)
            for m in range(KV_M):
                last["dve"] = nc.vector.tensor_mul(gsrc[:, m, :],
                                                   lat_ps[m], s_bc2)

            # k rot for own chunk (dup'd+perm'd cols of w_kva)
            kr_ps = psA.tile([128, CH], F32, tag="mm")
            for k in range(HID_K):
                last["pe"] = nc.tensor.matmul(
                    kr_ps, wkva_sb[:, k, KV_RANK:KV_RANK + 128],
                    ht_sb[:, k, :],
                    start=(k == 0), stop=(k == HID_K - 1))
            last["dve"] = rope(kr_ps, cosk_sb, sink_sb, gsrc[:, KV_M, :],
                               tmps, psX)

            # bounce + collective ride the gpsimd queue so the sync queue
            # (weight loads) never blocks behind them
            nc.gpsimd.dma_start(
                out=bounceA.rearrange("(t p) c -> p t c", p=128),
                in_=gsrc[:, 0:KV_M + 1, :])
            nc.gpsimd.collective_compute(
                "AllGather", mybir.AluOpType.bypass, replica_groups=RG,
                ins=[bounceA.opt()], outs=[gathA.opt()])

            # ---- q_a for own chunk (w_qa streamed per m-tile) ----
            sq_acc = tmps.tile([128, CH], F32R, tag="sqacc")
            for m in range(QA_M):
                wqa_m = wqa_pool.tile([128, HID_K, 128], BF16, tag="wqa")
                nc.sync.dma_start(
                    out=wqa_m,
                    in_=w_qa[:, m * 128:(m + 1) * 128]
                    .rearrange("(ko p) m -> p ko m", p=128))
                qa_ps = psA.tile([128, CH], F32, tag="mm")
                for k in range(HID_K):
                    last["pe"] = nc.tensor.matmul(
                        qa_ps, wqa_m[:, k, :],
                        ht_sb[:, k, :],
                        start=(k == 0), stop=(k == HID_K - 1))
                if m == 0:
                    last["act"] = nc.scalar.activation(sq_acc, qa_ps, AF.Square)
                else:
                    sq = tmps.tile([128, CH], F32R, tag="sq")
                    last["act"] = nc.scalar.activation(sq, qa_ps, AF.Square)
                    nc.gpsimd.tensor_tensor(sq_acc, sq_acc, sq, ALU.add)
                nc.scalar.copy(qa_sb[:, m, :], qa_ps)

            s_bc = rmsnorm_scale(sq_acc, sca, Q_RANK)
            for m in range(QA_M):
                last["dve"] = nc.vector.tensor_mul(gsrc[:, KV_M + 1 + m, :],
                                                   qa_sb[:, m, :], s_bc)

            # qa token-halves out + AllGathers (pipeline the CC stream)
            for hf in range(2):
                nc.gpsimd.dma_start(
                    out=bounceB[hf].rearrange("(t p) c -> p t c", p=128),
                    in_=gsrc[:, KV_M + 1:, hf * HC:(hf + 1) * HC])
                nc.gpsimd.collective_compute(
                    "AllGather", mybir.AluOpType.bypass, replica_groups=RG,
                    ins=[bounceB[hf].opt()], outs=[gathB[hf].opt()])

        # ============ PHASE G: kn/v, q_b + attention by token-half ========
        pg = ExitStack()
        with pg:
            gconst = pg.enter_context(tc.tile_pool(name="gconst", bufs=1))
            cos_sb = gconst.tile([128, S], F32)
            d_cos = nc.sync.dma_start(out=cos_sb, in_=cos2)
            sin_sb = gconst.tile([128, S], F32)
            d_sin = nc.sync.dma_start(out=sin_sb, in_=sin2)
            wqb_sb = gconst.tile([128, QA_M, HPC * D_QK], BF16)
            d_wqb = nc.sync.dma_start(
                out=wqb_sb, in_=w_qb.rearrange("(ko p) m -> p ko m", p=128))
            wkn_sb = gconst.tile([128, KV_M, HPC * D_NOPE], BF16)
            d_wkn = nc.sync.dma_start(
                out=wkn_sb, in_=w_kvb_n.rearrange("(ko p) m -> p ko m", p=128))
            wkv_sb = gconst.tile([128, KV_M, HPC * D_V], BF16)
            d_wkv = nc.sync.dma_start(
                out=wkv_sb, in_=w_kvb_v.rearrange("(ko p) m -> p ko m", p=128))
            wo_sb = gconst.tile([128, HPC, HID], BF16)
            d_wo = nc.sync.dma_start(
                out=wo_sb, in_=w_o.rearrange("(h p) n -> p h n", p=128))
            mask_sb = gconst.tile([128, 4, CH], F32)
            d_mk = nc.sync.dma_start(out=mask_sb,
                                     in_=masks.rearrange("v p q -> p v q"))

            gap = pg.enter_context(tc.tile_pool(name="gap", bufs=1))
            lat_all = gap.tile([128, KV_M, S], BF16)
            qap = pg.enter_context(tc.tile_pool(name="qap", bufs=2))
            d_g = []
            LKT = KV_M + 1  # tiles per rank in gathA
            for r in range(NCHUNK):
                base = r * LKT * 128
                rsl = slice(r * CH, (r + 1) * CH)
                d_g.append(nc.sync.dma_start(
                    out=lat_all[:, :, rsl],
                    in_=gathA[base:base + KV_M * 128, :]
                    .rearrange("(t p) c -> p t c", p=128)))
                d_g.append(nc.sync.dma_start(
                    out=krot_sb[:, rsl],
                    in_=gathA[base + KV_M * 128:base + LKT * 128, :]))

            fence = pe_observe(d_cos, d_sin, d_wqb, d_wkn, d_wkv, d_wo,
                               d_mk, *d_g, last.get("pe"), last.get("act"),
                               last.get("dve"))

            tmps = pg.enter_context(tc.tile_pool(name="tmpsG", bufs=2))
            expp = pg.enter_context(tc.tile_pool(name="expp", bufs=4))
            attnp = pg.enter_context(tc.tile_pool(name="attnp", bufs=8))
            outp = pg.enter_context(tc.tile_pool(name="outp", bufs=3))
            scr = pg.enter_context(tc.tile_pool(name="scr", bufs=2))
            psB = pg.enter_context(tc.tile_pool(name="psB", bufs=2, space="PSUM"))
            psDn = pg.enter_context(tc.tile_pool(name="psDn", bufs=1, space="PSUM"))
            psSc = pg.enter_context(tc.tile_pool(name="psSc", bufs=2, space="PSUM"))
            psAt = pg.enter_context(tc.tile_pool(name="psAt", bufs=2, space="PSUM"))

            # k_nope FM per head (only needs gathA -> overlaps B1/B2)
            for h in range(HPC):
                for r in range(NCHUNK):
                    rsl = slice(r * CH, (r + 1) * CH)
                    ps = psB.tile([128, CH], F32, tag="mm")
                    for k in range(KV_M):
                        mm = nc.tensor.matmul(
                            ps, wkn_sb[:, k, h * 128:(h + 1) * 128],
                            lat_all[:, k, rsl],
                            start=(k == 0), stop=(k == KV_M - 1))
                        if k == 0 and fence is not None:
                            add_dep_helper(mm.ins, fence.ins, False,
                                           "order after phase fence")
                            fence = None
                    last["dve"] = nc.vector.tensor_copy(
                        out=kn_sb[:, h, rsl], in_=ps)

            # V token-major: latent tiles stationary
            for tt in range(KT):
                ps = psB.tile([128, HPC * D_V], F32, tag="mm")
                for k in range(KV_M):
                    nc.tensor.matmul(
                        ps, lat_all[:, k, tt * 128:(tt + 1) * 128],
                        wkv_sb[:, k, :],
                        start=(k == 0), stop=(k == KV_M - 1))
                last["dve"] = nc.vector.tensor_copy(out=v_sb[:, tt, :],
                                                    in_=ps)

            def emit_qb(hf):
                """q_b + rope for token-half hf of every chunk."""
                for r in range(NCHUNK):
                    hsl = slice(r * CH + hf * HC, r * CH + (hf + 1) * HC)
                    base = r * QA_M * 128
                    qa_r = qap.tile([128, QA_M, HC], BF16, tag="qar")
                    nc.sync.dma_start(
                        out=qa_r,
                        in_=gathB[hf][base:base + QA_M * 128, :]
                        .rearrange("(t p) c -> p t c", p=128))
                    for m in range(QB_M):
                        qb_ps = psB.tile([128, HC], F32, tag="mm")
                        for k in range(QA_M):
                            nc.tensor.matmul(
                                qb_ps, wqb_sb[:, k, m * 128:(m + 1) * 128],
                                qa_r[:, k, :],
                                start=(k == 0), stop=(k == QA_M - 1))
                        if m < HPC:  # nope part: cast copy
                            last["act"] = nc.scalar.copy(
                                qpass_sb[:, m, hsl], qb_ps)
                        else:        # rot part
                            last["dve"] = rope(qb_ps, cos_sb[:, hsl],
                                               sin_sb[:, hsl],
                                               qrot_sb[:, m - HPC, hsl],
                                               tmps, psSc, width=HC)

            def emit_attn(hf):
                """attention + o-proj for token-half hf of every chunk."""
                for qc in range(NCHUNK):
                    hsl = slice(qc * CH + hf * HC, qc * CH + (hf + 1) * HC)
                    nkt = 4 * qc + 2 + 2 * hf
                    attn_tiles = []
                    for h in range(HPC):
                        j, par = h // 2, h % 2
                        poff = 64 * par
                        at_ps = psAt.tile([128, HC], F32, tag="attn")
                        dn_ps = psDn.tile([1, HC], F32, tag="denom")
                        for kt in range(nkt):
                            ksl = slice(kt * 128, (kt + 1) * 128)
                            sc_ps = psSc.tile([128, HC], F32, tag="scores")
                            nc.tensor.matmul(
                                sc_ps, kn_sb[:, h, ksl], qpass_sb[:, h, hsl],
                                start=True, stop=False)
                            nc.tensor.matmul(
                                sc_ps,
                                krot_sb[poff:poff + 64, ksl],
                                qrot_sb[poff:poff + 64, j, hsl],
                                start=False, stop=True)
                            if kt >= 4 * qc:  # diagonal block: causal mask
                                nc.vector.tensor_tensor(
                                    sc_ps, sc_ps,
                                    mask_sb[:, kt - 4 * qc,
                                            hf * HC:(hf + 1) * HC],
                                    ALU.add)
                            ex = expp.tile([128, HC], BF16, tag="exp")
                            nc.scalar.activation(ex, sc_ps, AF.Exp,
                                                 scale=SCALING)
                            nc.tensor.matmul(
                                at_ps, v_sb[:, kt, h * 128:(h + 1) * 128], ex,
                                start=(kt == 0), stop=(kt == nkt - 1))
                            nc.tensor.matmul(
                                dn_ps, ones_col_bf, ex,
                                start=(kt == 0), stop=(kt == nkt - 1))
                        # normalize: attn *= 1/denom
                        rc = scr.tile([1, HC], F32, tag="recip")
                        nc.vector.reciprocal(rc, dn_ps)
                        rb_sb = scr.tile([128, HC], F32, tag="rbs")
                        nc.gpsimd.partition_broadcast(rb_sb, rc, 128)
                        at_sb = attnp.tile([128, HC], BF16, tag="attn")
                        nc.vector.tensor_mul(at_sb, at_ps, rb_sb)
                        attn_tiles.append(at_sb)

                    # o-proj for this half-chunk
                    for tt in range(HC // 128):
                        for hck in range(NCHUNK):
                            o_ps = psB.tile([128, CH], F32, tag="mm")
                            for h in range(HPC):
                                nc.tensor.matmul(
                                    o_ps,
                                    attn_tiles[h][:, tt * 128:(tt + 1) * 128],
                                    wo_sb[:, h, hck * CH:(hck + 1) * CH],
                                    start=(h == 0), stop=(h == HPC - 1))
                            o_sb = outp.tile([128, CH], BF16, tag="osb")
                            nc.scalar.copy(o_sb, o_ps)
                            r0 = qc * CH + hf * HC + tt * 128
                            nc.sync.dma_start(
                                out=out[r0:r0 + 128, hck * CH:(hck + 1) * CH],
                                in_=o_sb)

            emit_qb(0)
            emit_attn(0)   # overlaps the B2 AllGather
            emit_qb(1)
            emit_attn(1)


# ---------------------------------------------------------------------------
# host side
# ---------------------------------------------------------------------------

_ILV = np.concatenate([np.arange(0, 64, 2), np.arange(1, 64, 2)])  # interleave


def _rot_half_mat():
    r = np.zeros((64, 64), np.float32)
    for m in range(32):
        r[m + 32, m] = -1.0
    for m in range(32, 64):
        r[m - 32, m] = 1.0
    r2 = np.zeros((128, 128), np.float32)
    r2[:64, :64] = r
    r2[64:, 64:] = r
    return r2


def _masks():
    mk = np.zeros((4, 128, CH), np.float32)
    i = np.arange(128)[:, None]
    j = np.arange(CH)[None, :]
    for v in range(4):
        mk[v] = np.where(v * 128 + i <= j, 0.0, NEG)
    return mk


def make_in_maps(hidden_states, w_qa, g_qa, w_qb, w_kva, g_kva, w_kvb, w_o,
                 cos, sin):
    bf = ml_dtypes.bfloat16
    w_qb_eff = (w_qb * g_qa[:, None]).astype(np.float32)
    w_kvb_eff = (w_kvb * g_kva[:, None]).astype(np.float32)
    r2 = _rot_half_mat()
    mk = _masks()

    in_maps = []
    for c in range(8):
        b, g = c // 4, c % 4
        heads = range(4 * g, 4 * g + 4)
        tsl = slice(g * CH, (g + 1) * CH)   # own token chunk

        hT = np.ascontiguousarray(hidden_states[b, tsl].T).astype(bf)

        # w_qb columns for these heads: 4 nope blocks then 2 rot tiles
        nope_cols = np.concatenate(
            [w_qb_eff[:, h * D_QK: h * D_QK + D_NOPE] for h in heads], axis=1)
        rot_cols = np.concatenate(
            [w_qb_eff[:, h * D_QK + D_NOPE: (h + 1) * D_QK][:, _ILV]
             for h in heads], axis=1)
        wqb_c = np.concatenate([nope_cols, rot_cols], axis=1).astype(bf)

        # w_kva: latent cols + interleaved rot cols duplicated
        rotw = w_kva[:, KV_RANK:KV_RANK + D_ROPE][:, _ILV]
        wkva_c = np.concatenate(
            [w_kva[:, :KV_RANK], rotw, rotw], axis=1).astype(bf)

        wkn_c = np.concatenate(
            [w_kvb_eff[:, h * (D_NOPE + D_V): h * (D_NOPE + D_V) + D_NOPE]
             for h in heads], axis=1).astype(bf)
        wkv_c = np.concatenate(
            [w_kvb_eff[:, h * (D_NOPE + D_V) + D_NOPE: (h + 1) * (D_NOPE + D_V)]
             for h in heads], axis=1).astype(bf)

        wo_c = np.concatenate(
            [w_o[h * D_V:(h + 1) * D_V, :] for h in heads], axis=0).astype(bf)

        cosT = np.ascontiguousarray(cos[b].T).astype(np.float32)  # [64, S]
        sinT = np.ascontiguousarray(sin[b].T).astype(np.float32)
        cos2 = np.concatenate([cosT, cosT], axis=0)
        sin2 = np.concatenate([sinT, sinT], axis=0)

        in_maps.append({
            "hT": hT,
            "w_qa": w_qa.astype(bf),
            "w_qb": wqb_c,
            "w_kva": wkva_c,
            "w_kvb_n": wkn_c,
            "w_kvb_v": wkv_c,
            "w_o": wo_c,
            "cos2": cos2,
            "sin2": sin2,
            "cosk": np.ascontiguousarray(cos2[:, tsl]),
            "sink": np.ascontiguousarray(sin2[:, tsl]),
            "r2": r2,
            "masks": mk,
        })
    return in_maps


_NC_CACHE = {}


def get_nc():
    if "nc" not in _NC_CACHE:
        _NC_CACHE["nc"] = build_nc()
    return _NC_CACHE["nc"]


def run(in_maps, **kw):
    nc = get_nc()
    return run_bass_kernel_spmd(nc, in_maps, list(range(8)), **kw)


def kernel(hidden_states, w_qa, g_qa, w_qb, w_kva, g_kva, w_kvb, w_o, cos, sin):
    args = [np.asarray(a) for a in (hidden_states, w_qa, g_qa, w_qb, w_kva,
                                    g_kva, w_kvb, w_o, cos, sin)]
    in_maps = make_in_maps(*args)
    res = run(in_maps).results
    out = np.zeros((B, S, HID), np.float32)
    for c in range(8):
        out[c // 4] += res[c]["out"].astype(np.float32)
    return out


# revision 15
# speedup vs baseline: 1.2688x; 1.2688x over previous
"""MLA attention (DeepSeek-style) Trainium2 Bass kernel.

Sharding: 8 cores = 2 batches x 4 head-groups (4 heads each). The shared
low-rank projections (q_a + rmsnorm, kv latent + rmsnorm, roped k_rot)
are token-sharded within each batch group: core g computes them for its
own 512-token chunk only, then 4-core DRAM AllGathers replicate the tiny
normalized latents. Attention + o-proj stay head-sharded (tensor
parallel); the host sums the 4 bf16 o-proj partials per batch.

Collective/compute overlap: the latent AllGather (A) is issued right
after the kv-latent matmuls and hides behind the q_a matmuls; the q_a
AllGather is split into two token-halves (B1/B2) that pipeline on the CC
stream. k_nope/V (need only A) overlap B1; attention for the first
token-half of every chunk (needs only B1) overlaps B2.

Layout convention on device: activations are feature-major "FM"
[feature on partitions, tokens on free dim]. Scores are [k, q] so that
softmax denominators / PV matmuls need no transposes anywhere.
"""

import sys
import numpy as np

sys.path.insert(0, "/opt/trn_rl_repo")

import ml_dtypes  # noqa: E402

import concourse.bass as bass  # noqa: E402
import concourse.bacc as bacc  # noqa: E402
import concourse.tile as tile  # noqa: E402
from concourse.tile_rust import add_dep_helper  # noqa: E402
from concourse import mybir  # noqa: E402
from concourse import bass_isa  # noqa: E402
from concourse.bass_utils import run_bass_kernel_spmd  # noqa: E402

F32 = mybir.dt.float32
F32R = mybir.dt.float32r
BF16 = mybir.dt.bfloat16
AF = mybir.ActivationFunctionType
ALU = mybir.AluOpType

# problem constants (hardcoded per contract)
B, S, HID = 2, 2048, 2048
H, D_NOPE, D_ROPE, D_V = 16, 128, 64, 128
D_QK = D_NOPE + D_ROPE
Q_RANK, KV_RANK = 1536, 512
EPS = 1e-6
SCALING = D_QK ** -0.5
NEG = -1.0e5  # causal mask additive constant (pre-scaling); exp -> 0

HPC = 4                      # heads per core
NCHUNK = 4                   # token chunks of 512
CH = S // NCHUNK             # 512
HC = CH // 2                 # 256 (attention token-half)
KT = S // 128                # 16 k tiles
QA_M = Q_RANK // 128         # 12
QB_M = (HPC * D_QK) // 128   # 6 (4 nope tiles + 2 rot tiles)
HID_K = HID // 128           # 16
KV_M = KV_RANK // 128        # 4
GT = QA_M + KV_M + 1         # 17 gather tiles: 4 latent + 1 krot + 12 qa
RG = [[0, 1, 2, 3], [4, 5, 6, 7]]


def build_nc():
    nc = bacc.Bacc(num_devices=8)

    # ---- I/O ----
    hT = nc.declare_dram_parameter("hT", [HID, CH], BF16, isOutput=False)
    w_qa = nc.declare_dram_parameter("w_qa", [HID, Q_RANK], BF16, isOutput=False)
    w_qb = nc.declare_dram_parameter("w_qb", [Q_RANK, HPC * D_QK], BF16, isOutput=False)
    w_kva = nc.declare_dram_parameter("w_kva", [HID, KV_RANK + 128], BF16, isOutput=False)
    w_kvb_n = nc.declare_dram_parameter("w_kvb_n", [KV_RANK, HPC * D_NOPE], BF16, isOutput=False)
    w_kvb_v = nc.declare_dram_parameter("w_kvb_v", [KV_RANK, HPC * D_V], BF16, isOutput=False)
    w_o = nc.declare_dram_parameter("w_o", [HPC * D_V, HID], BF16, isOutput=False)
    cos2 = nc.declare_dram_parameter("cos2", [128, S], F32, isOutput=False)
    sin2 = nc.declare_dram_parameter("sin2", [128, S], F32, isOutput=False)
    cosk = nc.declare_dram_parameter("cosk", [128, CH], F32, isOutput=False)
    sink = nc.declare_dram_parameter("sink", [128, CH], F32, isOutput=False)
    r2 = nc.declare_dram_parameter("r2", [128, 128], F32, isOutput=False)
    masks = nc.declare_dram_parameter("masks", [4, 128, CH], F32, isOutput=False)
    out = nc.declare_dram_parameter("out", [S, HID], BF16, isOutput=True)

    with tile.TileContext(nc) as tc:
        _emit(nc, tc, hT, w_qa, w_qb, w_kva, w_kvb_n, w_kvb_v, w_o,
              cos2, sin2, cosk, sink, r2, masks, out)
    nc.finalize()
    return nc


def _emit(nc, tc, hT, w_qa, w_qb, w_kva, w_kvb_n, w_kvb_v, w_o,
          cos2, sin2, cosk, sink, r2, masks, out):
    from contextlib import ExitStack

    (hT, w_qa, w_qb, w_kva, w_kvb_n, w_kvb_v, w_o, cos2, sin2, cosk, sink,
     r2, masks, out) = (
        x.ap() for x in (hT, w_qa, w_qb, w_kva, w_kvb_n, w_kvb_v, w_o,
                         cos2, sin2, cosk, sink, r2, masks, out))

    es = ExitStack()
    with es:
        # ------- tiny constants + long-lived activations -------
        tiny = es.enter_context(tc.tile_pool(name="tiny", bufs=1))
        ones_src = tiny.tile([128, 1], F32)
        nc.vector.memset(ones_src, 1.0)
        ones_col_bf = tiny.tile([128, 1], BF16)        # denom reducer lhsT
        nc.vector.memset(ones_col_bf, 1.0)
        ones_row_src = tiny.tile([1, 128], F32)
        nc.vector.memset(ones_row_src, 1.0)
        ones_row = tiny.tile([1, 128], F32R)            # fence lhsT
        nc.vector.tensor_copy(out=ones_row, in_=ones_row_src)
        eps_sb = tiny.tile([128, 1], F32)              # rmsnorm eps bias
        nc.vector.memset(eps_sb, EPS)
        r2_stage = tiny.tile([128, 128], F32)
        nc.sync.dma_start(out=r2_stage, in_=r2)
        r2_sb = tiny.tile([128, 128], F32R)
        nc.vector.tensor_copy(out=r2_sb, in_=r2_stage)  # DVE-produced (1-wait rule)

        psF = es.enter_context(tc.tile_pool(name="psF", bufs=1, space="PSUM"))
        fence_ps = psF.tile([1, 8], F32)

        # persistent activations consumed by attention
        qpass = es.enter_context(tc.tile_pool(name="qpass", bufs=1))
        qpass_sb = qpass.tile([128, HPC, S], BF16)     # qf nope, per head
        qrot_pool = es.enter_context(tc.tile_pool(name="qrot", bufs=1))
        qrot_sb = qrot_pool.tile([128, 2, S], BF16)    # qf rot, 2 heads per tile
        krot_pool = es.enter_context(tc.tile_pool(name="krot", bufs=1))
        krot_sb = krot_pool.tile([128, S], BF16)       # k rot (dup'd 64+64)
        kn_pool = es.enter_context(tc.tile_pool(name="kn", bufs=1))
        kn_sb = kn_pool.tile([128, HPC, S], BF16)
        v_pool = es.enter_context(tc.tile_pool(name="vtm", bufs=1))
        v_sb = v_pool.tile([128, KT, HPC * D_V], BF16)

        # DRAM bounce + gather buffers for the group AllGathers.
        dramp = es.enter_context(tc.tile_pool(name="dram", bufs=1, space="DRAM"))
        bounceA = dramp.tile([(KV_M + 1) * 128, CH], BF16)
        gathA = dramp.tile([NCHUNK * (KV_M + 1) * 128, CH], BF16)
        bounceB = [dramp.tile([QA_M * 128, HC], BF16, name=f"bounceB{i}")
                   for i in range(2)]
        gathB = [dramp.tile([NCHUNK * QA_M * 128, HC], BF16, name=f"gathB{i}")
                 for i in range(2)]

        last = {}  # most recent instruction handle per engine key

        def pe_observe(*deps):
            """Emit chained trivial PE matmuls, each sync-depending on one
            producer, so later PE matmuls don't accumulate multi-sem waits
            (fused-weight-load matmuls have few sync-wait slots in walrus).
            All write the same dedicated fence psum tile (same-engine WAW
            needs no semaphores). Returns the last absorber; phase-first
            matmuls must nosync-depend on it."""
            n = None
            for d in deps:
                if d is None:
                    continue
                prev = n
                n = nc.tensor.matmul(fence_ps[:, 0:8], ones_row[:, 0:1],
                                     ones_row[:, 0:8], start=True, stop=True,
                                     skip_group_check=True)
                add_dep_helper(n.ins, d.ins, True,
                               "phase-boundary PE observation")
                if prev is not None:
                    add_dep_helper(n.ins, prev.ins, False, "fence chain order")
            return n

        def rmsnorm_scale(sq_acc, sca, rank):
            """[128,CH] accumulated squares -> [128,CH] broadcast 1/rms."""
            ssq_bc = sca.tile([128, CH], F32, tag="ssqbc")
            nc.gpsimd.partition_all_reduce(ssq_bc, sq_acc, 128,
                                           bass_isa.ReduceOp.add)
            s_t = sca.tile([128, CH], F32, tag="srow")
            last["act"] = nc.scalar.activation(s_t, ssq_bc, AF.Sqrt,
                                               bias=eps_sb, scale=1.0 / rank)
            s_bc = sca.tile([128, CH], F32, tag="sbcs")
            last["dve"] = nc.vector.reciprocal(s_bc, s_t)
            return s_bc

        def rope(x_ps, cos_ap, sin_ap, out_ap, tmps, psX, width=CH):
            """RoPE a [128,width] psum tile (two 64-dim halves) -> out_ap."""
            xr = tmps.tile([128, width], F32R, tag="xr")
            nc.vector.tensor_copy(out=xr, in_=x_ps)
            rx_ps = psX.tile([128, width], F32, tag="scores")
            nc.tensor.matmul(rx_ps, r2_sb, xr)
            a_t = tmps.tile([128, width], F32, tag="ra")
            nc.vector.tensor_mul(a_t, xr, cos_ap)
            b_t = tmps.tile([128, width], F32, tag="rb")
            nc.vector.tensor_mul(b_t, rx_ps, sin_ap)
            return nc.vector.tensor_tensor(out_ap, a_t, b_t, ALU.add)

        # ============ PHASE L: local-chunk kv latent / k_rot / q_a ========
        pl = ExitStack()
        with pl:
            lconst = pl.enter_context(tc.tile_pool(name="lconst", bufs=1))
            ht_sb = lconst.tile([128, HID_K, CH], BF16)
            wkva_sb = lconst.tile([128, HID_K, KV_RANK + 128], BF16)
            # split the first loads so the first matmuls start sooner
            HK2 = HID_K // 2
            nc.sync.dma_start(
                out=ht_sb[:, 0:HK2, :],
                in_=hT[0:HK2 * 128, :].rearrange("(ko p) t -> p ko t", p=128))
            nc.sync.dma_start(
                out=wkva_sb[:, 0:HK2, :],
                in_=w_kva[0:HK2 * 128, :].rearrange("(ko p) m -> p ko m", p=128))
            nc.sync.dma_start(
                out=ht_sb[:, HK2:, :],
                in_=hT[HK2 * 128:, :].rearrange("(ko p) t -> p ko t", p=128))
            nc.sync.dma_start(
                out=wkva_sb[:, HK2:, :],
                in_=w_kva[HK2 * 128:, :].rearrange("(ko p) m -> p ko m", p=128))
            cosk_sb = lconst.tile([128, CH], F32)
            nc.sync.dma_start(out=cosk_sb, in_=cosk)
            sink_sb = lconst.tile([128, CH], F32)
            nc.sync.dma_start(out=sink_sb, in_=sink)
            wqa_pool = pl.enter_context(tc.tile_pool(name="wqa", bufs=2))

            qa_st = pl.enter_context(tc.tile_pool(name="qast", bufs=1))
            qa_sb = qa_st.tile([128, QA_M, CH], F32)
            gsrcp = pl.enter_context(tc.tile_pool(name="gsrc", bufs=1))
            gsrc = gsrcp.tile([128, GT, CH], BF16)

            tmps = pl.enter_context(tc.tile_pool(name="tmpsL", bufs=2))
            sca = pl.enter_context(tc.tile_pool(name="scaleL", bufs=2))
            psA = pl.enter_context(tc.tile_pool(name="psA", bufs=2, space="PSUM"))
            psX = pl.enter_context(tc.tile_pool(name="psX", bufs=1, space="PSUM"))
            psLat = pl.enter_context(tc.tile_pool(name="psLat", bufs=1, space="PSUM"))

            # ---- kv latent for own chunk (feeds the early AllGather) ----
            sq_acc2 = tmps.tile([128, CH], F32R, tag="sqacc2")
            lat_ps = []
            for m in range(KV_M):
                l_ps = psLat.tile([128, CH], F32, tag=f"lat{m}")
                lat_ps.append(l_ps)
                for k in range(HID_K):
                    last["pe"] = nc.tensor.matmul(
                        l_ps, wkva_sb[:, k, m * 128:(m + 1) * 128],
                        ht_sb[:, k, :],
                        start=(k == 0), stop=(k == HID_K - 1))
                if m == 0:
                    last["act"] = nc.scalar.activation(sq_acc2, l_ps, AF.Square)
                else:
                    sq = tmps.tile([128, CH], F32R, tag="sq")
                    last["act"] = nc.scalar.activation(sq, l_ps, AF.Square)
                    nc.gpsimd.tensor_tensor(sq_acc2, sq_acc2, sq, ALU.add)

            s_bc2 = rmsnorm_scale(sq_acc2, sca, KV_RANK)
            for m in range(KV_M):
                last["dve"] = nc.vector.tensor_mul(gsrc[:, m, :],
                                                   lat_ps[m], s_bc2)

            # k rot for own chunk (dup'd+perm'd cols of w_kva)
            kr_ps = psA.tile([128, CH], F32, tag="mm")
            for k in range(HID_K):
                last["pe"] = nc.tensor.matmul(
                    kr_ps, wkva_sb[:, k, KV_RANK:KV_RANK + 128],
                    ht_sb[:, k, :],
                    start=(k == 0), stop=(k == HID_K - 1))
            last["dve"] = rope(kr_ps, cosk_sb, sink_sb, gsrc[:, KV_M, :],
                               tmps, psX)

            # bounce + collective ride the gpsimd queue so the sync queue
            # (weight loads) never blocks behind them
            nc.gpsimd.dma_start(
                out=bounceA.rearrange("(t p) c -> p t c", p=128),
                in_=gsrc[:, 0:KV_M + 1, :])
            nc.gpsimd.collective_compute(
                "AllGather", mybir.AluOpType.bypass, replica_groups=RG,
                ins=[bounceA.opt()], outs=[gathA.opt()])

            # ---- q_a for own chunk (w_qa streamed per m-tile) ----
            sq_acc = tmps.tile([128, CH], F32R, tag="sqacc")
            for m in range(QA_M):
                wqa_m = wqa_pool.tile([128, HID_K, 128], BF16, tag="wqa")
                nc.sync.dma_start(
                    out=wqa_m,
                    in_=w_qa[:, m * 128:(m + 1) * 128]
                    .rearrange("(ko p) m -> p ko m", p=128))
                qa_ps = psA.tile([128, CH], F32, tag="mm")
                for k in range(HID_K):
                    last["pe"] = nc.tensor.matmul(
                        qa_ps, wqa_m[:, k, :],
                        ht_sb[:, k, :],
                        start=(k == 0), stop=(k == HID_K - 1))
                if m == 0:
                    last["act"] = nc.scalar.activation(sq_acc, qa_ps, AF.Square)
                else:
                    sq = tmps.tile([128, CH], F32R, tag="sq")
                    last["act"] = nc.scalar.activation(sq, qa_ps, AF.Square)
                    nc.gpsimd.tensor_tensor(sq_acc, sq_acc, sq, ALU.add)
                nc.scalar.copy(qa_sb[:, m, :], qa_ps)

            s_bc = rmsnorm_scale(sq_acc, sca, Q_RANK)
            for m in range(QA_M):
                last["dve"] = nc.vector.tensor_mul(gsrc[:, KV_M + 1 + m, :],
                                                   qa_sb[:, m, :], s_bc)

            # qa token-halves out + AllGathers (pipeline the CC stream)
            for hf in range(2):
                nc.gpsimd.dma_start(
                    out=bounceB[hf].rearrange("(t p) c -> p t c", p=128),
                    in_=gsrc[:, KV_M + 1:, hf * HC:(hf + 1) * HC])
                nc.gpsimd.collective_compute(
                    "AllGather", mybir.AluOpType.bypass, replica_groups=RG,
                    ins=[bounceB[hf].opt()], outs=[gathB[hf].opt()])

        # ============ PHASE G: kn/v, q_b + attention by token-half ========
        pg = ExitStack()
        with pg:
            gconst = pg.enter_context(tc.tile_pool(name="gconst", bufs=1))
            cos_sb = gconst.tile([128, S], F32)
            d_cos = nc.sync.dma_start(out=cos_sb, in_=cos2)
            sin_sb = gconst.tile([128, S], F32)
            d_sin = nc.sync.dma_start(out=sin_sb, in_=sin2)
            wqb_sb = gconst.tile([128, QA_M, HPC * D_QK], BF16)
            d_wqb = nc.sync.dma_start(
                out=wqb_sb, in_=w_qb.rearrange("(ko p) m -> p ko m", p=128))
            wkn_sb = gconst.tile([128, KV_M, HPC * D_NOPE], BF16)
            d_wkn = nc.sync.dma_start(
                out=wkn_sb, in_=w_kvb_n.rearrange("(ko p) m -> p ko m", p=128))
            wkv_sb = gconst.tile([128, KV_M, HPC * D_V], BF16)
            d_wkv = nc.sync.dma_start(
                out=wkv_sb, in_=w_kvb_v.rearrange("(ko p) m -> p ko m", p=128))
            wo_sb = gconst.tile([128, HPC, HID], BF16)
            d_wo = nc.sync.dma_start(
                out=wo_sb, in_=w_o.rearrange("(h p) n -> p h n", p=128))
            mask_sb = gconst.tile([128, 4, CH], F32)
            d_mk = nc.sync.dma_start(out=mask_sb,
                                     in_=masks.rearrange("v p q -> p v q"))

            gap = pg.enter_context(tc.tile_pool(name="gap", bufs=1))
            lat_all = gap.tile([128, KV_M, S], BF16)
            qap = pg.enter_context(tc.tile_pool(name="qap", bufs=2))
            d_g = []
            LKT = KV_M + 1  # tiles per rank in gathA
            for r in range(NCHUNK):
                base = r * LKT * 128
                rsl = slice(r * CH, (r + 1) * CH)
                d_g.append(nc.sync.dma_start(
                    out=lat_all[:, :, rsl],
                    in_=gathA[base:base + KV_M * 128, :]
                    .rearrange("(t p) c -> p t c", p=128)))
                d_g.append(nc.sync.dma_start(
                    out=krot_sb[:, rsl],
                    in_=gathA[base + KV_M * 128:base + LKT * 128, :]))

            fence = pe_observe(d_cos, d_sin, d_wqb, d_wkn, d_wkv, d_wo,
                               d_mk, *d_g, last.get("pe"), last.get("act"),
                               last.get("dve"))

            tmps = pg.enter_context(tc.tile_pool(name="tmpsG", bufs=2))
            expp = pg.enter_context(tc.tile_pool(name="expp", bufs=4))
            attnp = pg.enter_context(tc.tile_pool(name="attnp", bufs=8))
            outp = pg.enter_context(tc.tile_pool(name="outp", bufs=3))
            scr = pg.enter_context(tc.tile_pool(name="scr", bufs=2))
            psB = pg.enter_context(tc.tile_pool(name="psB", bufs=2, space="PSUM"))
            psDn = pg.enter_context(tc.tile_pool(name="psDn", bufs=1, space="PSUM"))
            psSc = pg.enter_context(tc.tile_pool(name="psSc", bufs=2, space="PSUM"))
            psAt = pg.enter_context(tc.tile_pool(name="psAt", bufs=2, space="PSUM"))

            # k_nope FM per head (only needs gathA -> overlaps B1/B2)
            for h in range(HPC):
                for r in range(NCHUNK):
                    rsl = slice(r * CH, (r + 1) * CH)
                    ps = psB.tile([128, CH], F32, tag="mm")
                    for k in range(KV_M):
                        mm = nc.tensor.matmul(
                            ps, wkn_sb[:, k, h * 128:(h + 1) * 128],
                            lat_all[:, k, rsl],
                            start=(k == 0), stop=(k == KV_M - 1))
                        if k == 0 and fence is not None:
                            add_dep_helper(mm.ins, fence.ins, False,
                                           "order after phase fence")
                            fence = None
                    last["dve"] = nc.vector.tensor_copy(
                        out=kn_sb[:, h, rsl], in_=ps)

            # V token-major: latent tiles stationary
            for tt in range(KT):
                ps = psB.tile([128, HPC * D_V], F32, tag="mm")
                for k in range(KV_M):
                    nc.tensor.matmul(
                        ps, lat_all[:, k, tt * 128:(tt + 1) * 128],
                        wkv_sb[:, k, :],
                        start=(k == 0), stop=(k == KV_M - 1))
                last["dve"] = nc.vector.tensor_copy(out=v_sb[:, tt, :],
                                                    in_=ps)

            def emit_qb(hf):
                """q_b + rope for token-half hf of every chunk."""
                for r in range(NCHUNK):
                    hsl = slice(r * CH + hf * HC, r * CH + (hf + 1) * HC)
                    base = r * QA_M * 128
                    qa_r = qap.tile([128, QA_M, HC], BF16, tag="qar")
                    nc.sync.dma_start(
                        out=qa_r,
                        in_=gathB[hf][base:base + QA_M * 128, :]
                        .rearrange("(t p) c -> p t c", p=128))
                    for m in range(QB_M):
                        qb_ps = psB.tile([128, HC], F32, tag="mm")
                        for k in range(QA_M):
                            nc.tensor.matmul(
                                qb_ps, wqb_sb[:, k, m * 128:(m + 1) * 128],
                                qa_r[:, k, :],
                                start=(k == 0), stop=(k == QA_M - 1))
                        if m < HPC:  # nope part: cast copy
                            last["act"] = nc.scalar.copy(
                                qpass_sb[:, m, hsl], qb_ps)
                        else:        # rot part
                            last["dve"] = rope(qb_ps, cos_sb[:, hsl],
                                               sin_sb[:, hsl],
                                               qrot_sb[:, m - HPC, hsl],
                                               tmps, psSc, width=HC)

            def emit_attn():
                """attention + o-proj, full 512-token chunks."""
                for qc in range(NCHUNK):
                    csl = slice(qc * CH, (qc + 1) * CH)
                    nkt = 4 * qc + 4
                    attn_tiles = []
                    for h in range(HPC):
                        j, par = h // 2, h % 2
                        poff = 64 * par
                        at_ps = psAt.tile([128, CH], F32, tag="attn")
                        dn_ps = psDn.tile([1, CH], F32, tag="denom")
                        for kt in range(nkt):
                            ksl = slice(kt * 128, (kt + 1) * 128)
                            sc_ps = psSc.tile([128, CH], F32, tag="scores")
                            nc.tensor.matmul(
                                sc_ps, kn_sb[:, h, ksl], qpass_sb[:, h, csl],
                                start=True, stop=False)
                            nc.tensor.matmul(
                                sc_ps,
                                krot_sb[poff:poff + 64, ksl],
                                qrot_sb[poff:poff + 64, j, csl],
                                start=False, stop=True)
                            if kt >= 4 * qc:  # diagonal block: causal mask
                                nc.vector.tensor_tensor(
                                    sc_ps, sc_ps, mask_sb[:, kt - 4 * qc, :],
                                    ALU.add)
                            ex = expp.tile([128, CH], BF16, tag="exp")
                            nc.scalar.activation(ex, sc_ps, AF.Exp,
                                                 scale=SCALING)
                            nc.tensor.matmul(
                                at_ps, v_sb[:, kt, h * 128:(h + 1) * 128], ex,
                                start=(kt == 0), stop=(kt == nkt - 1))
                            nc.tensor.matmul(
                                dn_ps, ones_col_bf, ex,
                                start=(kt == 0), stop=(kt == nkt - 1))
                        # normalize: attn *= 1/denom
                        rc = scr.tile([1, CH], F32, tag="recip")
                        nc.vector.reciprocal(rc, dn_ps)
                        rb_sb = scr.tile([128, CH], F32, tag="rbs")
                        nc.gpsimd.partition_broadcast(rb_sb, rc, 128)
                        at_sb = attnp.tile([128, CH], BF16, tag="attn")
                        nc.vector.tensor_mul(at_sb, at_ps, rb_sb)
                        attn_tiles.append(at_sb)

                    # o-proj for this chunk
                    for tt in range(CH // 128):
                        for hck in range(NCHUNK):
                            o_ps = psB.tile([128, CH], F32, tag="mm")
                            for h in range(HPC):
                                nc.tensor.matmul(
                                    o_ps,
                                    attn_tiles[h][:, tt * 128:(tt + 1) * 128],
                                    wo_sb[:, h, hck * CH:(hck + 1) * CH],
                                    start=(h == 0), stop=(h == HPC - 1))
                            o_sb = outp.tile([128, CH], BF16, tag="osb")
                            nc.scalar.copy(o_sb, o_ps)
                            r0 = qc * CH + tt * 128
                            nc.sync.dma_start(
                                out=out[r0:r0 + 128, hck * CH:(hck + 1) * CH],
                                in_=o_sb)

            emit_qb(0)
            emit_qb(1)
            emit_attn()


# ---------------------------------------------------------------------------
# host side
# ---------------------------------------------------------------------------

_ILV = np.concatenate([np.arange(0, 64, 2), np.arange(1, 64, 2)])  # interleave


def _rot_half_mat():
    r = np.zeros((64, 64), np.float32)
    for m in range(32):
        r[m + 32, m] = -1.0
    for m in range(32, 64):
        r[m - 32, m] = 1.0
    r2 = np.zeros((128, 128), np.float32)
    r2[:64, :64] = r
    r2[64:, 64:] = r
    return r2


def _masks():
    mk = np.zeros((4, 128, CH), np.float32)
    i = np.arange(128)[:, None]
    j = np.arange(CH)[None, :]
    for v in range(4):
        mk[v] = np.where(v * 128 + i <= j, 0.0, NEG)
    return mk


def make_in_maps(hidden_states, w_qa, g_qa, w_qb, w_kva, g_kva, w_kvb, w_o,
                 cos, sin):
    bf = ml_dtypes.bfloat16
    w_qb_eff = (w_qb * g_qa[:, None]).astype(np.float32)
    w_kvb_eff = (w_kvb * g_kva[:, None]).astype(np.float32)
    r2 = _rot_half_mat()
    mk = _masks()

    in_maps = []
    for c in range(8):
        b, g = c // 4, c % 4
        heads = range(4 * g, 4 * g + 4)
        tsl = slice(g * CH, (g + 1) * CH)   # own token chunk

        hT = np.ascontiguousarray(hidden_states[b, tsl].T).astype(bf)

        # w_qb columns for these heads: 4 nope blocks then 2 rot tiles
        nope_cols = np.concatenate(
            [w_qb_eff[:, h * D_QK: h * D_QK + D_NOPE] for h in heads], axis=1)
        rot_cols = np.concatenate(
            [w_qb_eff[:, h * D_QK + D_NOPE: (h + 1) * D_QK][:, _ILV]
             for h in heads], axis=1)
        wqb_c = np.concatenate([nope_cols, rot_cols], axis=1).astype(bf)

        # w_kva: latent cols + interleaved rot cols duplicated
        rotw = w_kva[:, KV_RANK:KV_RANK + D_ROPE][:, _ILV]
        wkva_c = np.concatenate(
            [w_kva[:, :KV_RANK], rotw, rotw], axis=1).astype(bf)

        wkn_c = np.concatenate(
            [w_kvb_eff[:, h * (D_NOPE + D_V): h * (D_NOPE + D_V) + D_NOPE]
             for h in heads], axis=1).astype(bf)
        wkv_c = np.concatenate(
            [w_kvb_eff[:, h * (D_NOPE + D_V) + D_NOPE: (h + 1) * (D_NOPE + D_V)]
             for h in heads], axis=1).astype(bf)

        wo_c = np.concatenate(
            [w_o[h * D_V:(h + 1) * D_V, :] for h in heads], axis=0).astype(bf)

        cosT = np.ascontiguousarray(cos[b].T).astype(np.float32)  # [64, S]
        sinT = np.ascontiguousarray(sin[b].T).astype(np.float32)
        cos2 = np.concatenate([cosT, cosT], axis=0)
        sin2 = np.concatenate([sinT, sinT], axis=0)

        in_maps.append({
            "hT": hT,
            "w_qa": w_qa.astype(bf),
            "w_qb": wqb_c,
            "w_kva": wkva_c,
            "w_kvb_n": wkn_c,
            "w_kvb_v": wkv_c,
            "w_o": wo_c,
            "cos2": cos2,
            "sin2": sin2,
            "cosk": np.ascontiguousarray(cos2[:, tsl]),
            "sink": np.ascontiguousarray(sin2[:, tsl]),
            "r2": r2,
            "masks": mk,
        })
    return in_maps


_NC_CACHE = {}


def get_nc():
    if "nc" not in _NC_CACHE:
        _NC_CACHE["nc"] = build_nc()
    return _NC_CACHE["nc"]


def run(in_maps, **kw):
    nc = get_nc()
    return run_bass_kernel_spmd(nc, in_maps, list(range(8)), **kw)


def kernel(hidden_states, w_qa, g_qa, w_qb, w_kva, g_kva, w_kvb, w_o, cos, sin):
    args = [np.asarray(a) for a in (hidden_states, w_qa, g_qa, w_qb, w_kva,
                                    g_kva, w_kvb, w_o, cos, sin)]
    in_maps = make_in_maps(*args)
    res = run(in_maps).results
    out = np.zeros((B, S, HID), np.float32)
    for c in range(8):
        out[c // 4] += res[c]["out"].astype(np.float32)
    return out


# revision 19
# speedup vs baseline: 1.3157x; 1.0370x over previous
"""MLA attention (DeepSeek-style) Trainium2 Bass kernel.

Sharding: 8 cores = 2 batches x 4 head-groups (4 heads each). The shared
low-rank projections (q_a + rmsnorm, kv latent + rmsnorm, roped k_rot)
are token-sharded within each batch group: core g computes them for its
own 512-token chunk only, then 4-core DRAM AllGathers replicate the tiny
normalized latents. Attention + o-proj stay head-sharded (tensor
parallel); the host sums the 4 bf16 o-proj partials per batch.

Collective/compute overlap: the latent AllGather (A) is issued right
after the kv-latent matmuls and hides behind the q_a matmuls; the q_a
AllGather is split into two token-halves (B1/B2) that pipeline on the CC
stream. k_nope/V (need only A) overlap B1; attention for the first
token-half of every chunk (needs only B1) overlaps B2.

Layout convention on device: activations are feature-major "FM"
[feature on partitions, tokens on free dim]. Scores are [k, q] so that
softmax denominators / PV matmuls need no transposes anywhere.
"""

import sys
import numpy as np

sys.path.insert(0, "/opt/trn_rl_repo")

import ml_dtypes  # noqa: E402

import concourse.bass as bass  # noqa: E402
import concourse.bacc as bacc  # noqa: E402
import concourse.tile as tile  # noqa: E402
from concourse.tile_rust import add_dep_helper  # noqa: E402
from concourse import mybir  # noqa: E402
from concourse import bass_isa  # noqa: E402
from concourse.bass_utils import run_bass_kernel_spmd  # noqa: E402

F32 = mybir.dt.float32
F32R = mybir.dt.float32r
BF16 = mybir.dt.bfloat16
AF = mybir.ActivationFunctionType
ALU = mybir.AluOpType

# problem constants (hardcoded per contract)
B, S, HID = 2, 2048, 2048
H, D_NOPE, D_ROPE, D_V = 16, 128, 64, 128
D_QK = D_NOPE + D_ROPE
Q_RANK, KV_RANK = 1536, 512
EPS = 1e-6
SCALING = D_QK ** -0.5
NEG = -1.0e5  # causal mask additive constant (pre-scaling); exp -> 0

HPC = 4                      # heads per core
NCHUNK = 4                   # token chunks of 512
CH = S // NCHUNK             # 512
HC = CH // 2                 # 256 (attention token-half)
KT = S // 128                # 16 k tiles
QA_M = Q_RANK // 128         # 12
QB_M = (HPC * D_QK) // 128   # 6 (4 nope tiles + 2 rot tiles)
HID_K = HID // 128           # 16
KV_M = KV_RANK // 128        # 4
GT = QA_M + KV_M + 1         # 17 gather tiles: 4 latent + 1 krot + 12 qa
RG = [[0, 1, 2, 3], [4, 5, 6, 7]]


def build_nc():
    nc = bacc.Bacc(num_devices=8)

    # ---- I/O ----
    hT = nc.declare_dram_parameter("hT", [128, HID_K, CH], BF16, isOutput=False)
    w_qa = nc.declare_dram_parameter("w_qa", [128, QA_M, HID_K, 128], BF16, isOutput=False)
    w_qb = nc.declare_dram_parameter("w_qb", [128, QA_M, HPC * D_QK], BF16, isOutput=False)
    w_kva = nc.declare_dram_parameter("w_kva", [128, HID_K, KV_RANK + 128], BF16, isOutput=False)
    w_kvb_n = nc.declare_dram_parameter("w_kvb_n", [128, KV_M, HPC * D_NOPE], BF16, isOutput=False)
    w_kvb_v = nc.declare_dram_parameter("w_kvb_v", [128, KV_M, HPC * D_V], BF16, isOutput=False)
    w_o = nc.declare_dram_parameter("w_o", [128, HPC, HID], BF16, isOutput=False)
    cos2 = nc.declare_dram_parameter("cos2", [128, S], F32, isOutput=False)
    sin2 = nc.declare_dram_parameter("sin2", [128, S], F32, isOutput=False)
    cosk = nc.declare_dram_parameter("cosk", [128, CH], F32, isOutput=False)
    sink = nc.declare_dram_parameter("sink", [128, CH], F32, isOutput=False)
    r2 = nc.declare_dram_parameter("r2", [128, 128], F32, isOutput=False)
    masks = nc.declare_dram_parameter("masks", [128, 4, CH], F32, isOutput=False)
    out = nc.declare_dram_parameter("out", [S, HID], BF16, isOutput=True)

    with tile.TileContext(nc) as tc:
        _emit(nc, tc, hT, w_qa, w_qb, w_kva, w_kvb_n, w_kvb_v, w_o,
              cos2, sin2, cosk, sink, r2, masks, out)
    nc.finalize()
    return nc


def _emit(nc, tc, hT, w_qa, w_qb, w_kva, w_kvb_n, w_kvb_v, w_o,
          cos2, sin2, cosk, sink, r2, masks, out):
    from contextlib import ExitStack

    (hT, w_qa, w_qb, w_kva, w_kvb_n, w_kvb_v, w_o, cos2, sin2, cosk, sink,
     r2, masks, out) = (
        x.ap() for x in (hT, w_qa, w_qb, w_kva, w_kvb_n, w_kvb_v, w_o,
                         cos2, sin2, cosk, sink, r2, masks, out))

    es = ExitStack()
    with es:
        # ------- tiny constants + long-lived activations -------
        tiny = es.enter_context(tc.tile_pool(name="tiny", bufs=1))
        ones_src = tiny.tile([128, 1], F32)
        nc.vector.memset(ones_src, 1.0)
        ones_col_bf = tiny.tile([128, 1], BF16)        # denom reducer lhsT
        nc.vector.memset(ones_col_bf, 1.0)
        ones_row_src = tiny.tile([1, 128], F32)
        nc.vector.memset(ones_row_src, 1.0)
        ones_row = tiny.tile([1, 128], F32R)            # fence lhsT
        nc.vector.tensor_copy(out=ones_row, in_=ones_row_src)
        eps_sb = tiny.tile([128, 1], F32)              # rmsnorm eps bias
        nc.vector.memset(eps_sb, EPS)
        r2_stage = tiny.tile([128, 128], F32)
        nc.sync.dma_start(out=r2_stage, in_=r2)
        r2_sb = tiny.tile([128, 128], F32R)
        nc.vector.tensor_copy(out=r2_sb, in_=r2_stage)  # DVE-produced (1-wait rule)

        psF = es.enter_context(tc.tile_pool(name="psF", bufs=1, space="PSUM"))
        fence_ps = psF.tile([1, 8], F32)

        # persistent activations consumed by attention
        qpass = es.enter_context(tc.tile_pool(name="qpass", bufs=1))
        qpass_sb = qpass.tile([128, HPC, S], BF16)     # qf nope, per head
        qrot_pool = es.enter_context(tc.tile_pool(name="qrot", bufs=1))
        qrot_sb = qrot_pool.tile([128, 2, S], BF16)    # qf rot, 2 heads per tile
        krot_pool = es.enter_context(tc.tile_pool(name="krot", bufs=1))
        krot_sb = krot_pool.tile([128, S], BF16)       # k rot (dup'd 64+64)
        kn_pool = es.enter_context(tc.tile_pool(name="kn", bufs=1))
        kn_sb = kn_pool.tile([128, HPC, S], BF16)
        v_pool = es.enter_context(tc.tile_pool(name="vtm", bufs=1))
        v_sb = v_pool.tile([128, KT, HPC * D_V], BF16)
        lat_pool = es.enter_context(tc.tile_pool(name="latp", bufs=1))
        lat_all = lat_pool.tile([128, KV_M, S], BF16)
        wkn_sb = lat_pool.tile([128, KV_M, HPC * D_NOPE], BF16)
        wkv_sb = lat_pool.tile([128, KV_M, HPC * D_V], BF16)

        # DRAM bounce + gather buffers for the group AllGathers.
        dramp = es.enter_context(tc.tile_pool(name="dram", bufs=1, space="DRAM"))
        bounceA = dramp.tile([128, KV_M + 1, CH], BF16)
        gathA = dramp.tile([NCHUNK, 128, KV_M + 1, CH], BF16)
        bounceB = [dramp.tile([128, QA_M, HC], BF16, name=f"bounceB{i}")
                   for i in range(2)]
        gathB = [dramp.tile([NCHUNK, 128, QA_M, HC], BF16, name=f"gathB{i}")
                 for i in range(2)]

        last = {}  # most recent instruction handle per engine key

        def pe_observe(*deps):
            """Emit chained trivial PE matmuls, each sync-depending on one
            producer, so later PE matmuls don't accumulate multi-sem waits
            (fused-weight-load matmuls have few sync-wait slots in walrus).
            All write the same dedicated fence psum tile (same-engine WAW
            needs no semaphores). Returns the last absorber; phase-first
            matmuls must nosync-depend on it."""
            n = None
            for d in deps:
                if d is None:
                    continue
                prev = n
                n = nc.tensor.matmul(fence_ps[:, 0:8], ones_row[:, 0:1],
                                     ones_row[:, 0:8], start=True, stop=True,
                                     skip_group_check=True)
                add_dep_helper(n.ins, d.ins, True,
                               "phase-boundary PE observation")
                if prev is not None:
                    add_dep_helper(n.ins, prev.ins, False, "fence chain order")
            return n

        def rmsnorm_scale(sq_acc, sca, rank):
            """[128,CH] accumulated squares -> [128,CH] broadcast 1/rms."""
            ssq_bc = sca.tile([128, CH], F32, tag="ssqbc")
            nc.gpsimd.partition_all_reduce(ssq_bc, sq_acc, 128,
                                           bass_isa.ReduceOp.add)
            s_t = sca.tile([128, CH], F32, tag="srow")
            last["act"] = nc.scalar.activation(s_t, ssq_bc, AF.Sqrt,
                                               bias=eps_sb, scale=1.0 / rank)
            s_bc = sca.tile([128, CH], F32, tag="sbcs")
            last["dve"] = nc.vector.reciprocal(s_bc, s_t)
            return s_bc

        def rope(x_ps, cos_ap, sin_ap, out_ap, tmps, psX, width=CH):
            """RoPE a [128,width] psum tile (two 64-dim halves) -> out_ap."""
            xr = tmps.tile([128, width], F32R, tag="xr")
            nc.vector.tensor_copy(out=xr, in_=x_ps)
            rx_ps = psX.tile([128, width], F32, tag="scores")
            nc.tensor.matmul(rx_ps, r2_sb, xr)
            a_t = tmps.tile([128, width], F32, tag="ra")
            nc.vector.tensor_mul(a_t, xr, cos_ap)
            b_t = tmps.tile([128, width], F32, tag="rb")
            nc.vector.tensor_mul(b_t, rx_ps, sin_ap)
            return nc.vector.tensor_tensor(out_ap, a_t, b_t, ALU.add)

        # ============ PHASE L: local-chunk kv latent / k_rot / q_a ========
        pl = ExitStack()
        with pl:
            lconst = pl.enter_context(tc.tile_pool(name="lconst", bufs=1))
            ht_sb = lconst.tile([128, HID_K, CH], BF16)
            wkva_sb = lconst.tile([128, HID_K, KV_RANK + 128], BF16)
            # split the first loads so the first matmuls start sooner
            for k in range(HID_K):
                nc.sync.dma_start(out=ht_sb[:, k, :], in_=hT[:, k, :])
                nc.sync.dma_start(out=wkva_sb[:, k, :], in_=w_kva[:, k, :])
            cosk_sb = lconst.tile([128, CH], F32)
            nc.sync.dma_start(out=cosk_sb, in_=cosk)
            sink_sb = lconst.tile([128, CH], F32)
            nc.sync.dma_start(out=sink_sb, in_=sink)
            wqa_pool = pl.enter_context(tc.tile_pool(name="wqa", bufs=2))

            qa_st = pl.enter_context(tc.tile_pool(name="qast", bufs=1))
            qa_sb = qa_st.tile([128, QA_M, CH], BF16)
            gsrcp = pl.enter_context(tc.tile_pool(name="gsrc", bufs=1))
            gsrc = gsrcp.tile([128, GT, CH], BF16)

            tmps = pl.enter_context(tc.tile_pool(name="tmpsL", bufs=2))
            sca = pl.enter_context(tc.tile_pool(name="scaleL", bufs=2))
            psA = pl.enter_context(tc.tile_pool(name="psA", bufs=2, space="PSUM"))
            psX = pl.enter_context(tc.tile_pool(name="psX", bufs=1, space="PSUM"))
            psLat = pl.enter_context(tc.tile_pool(name="psLat", bufs=1, space="PSUM"))

            # ---- kv latent + k_rot, k-outer so PE starts on the first
            # ht/wkva k-tile DMA instead of the full 4.6MB load ----
            sq_acc2 = tmps.tile([128, CH], F32R, tag="sqacc2")
            lat_ps = []
            for m in range(KV_M):
                l_ps = psLat.tile([128, CH], F32, tag=f"lat{m}")
                lat_ps.append(l_ps)
            kr_ps = psA.tile([128, CH], F32, tag="mm")
            for k in range(HID_K):
                for m in range(KV_M):
                    last["pe"] = nc.tensor.matmul(
                        lat_ps[m], wkva_sb[:, k, m * 128:(m + 1) * 128],
                        ht_sb[:, k, :],
                        start=(k == 0), stop=(k == HID_K - 1))
                last["pe"] = nc.tensor.matmul(
                    kr_ps, wkva_sb[:, k, KV_RANK:KV_RANK + 128],
                    ht_sb[:, k, :],
                    start=(k == 0), stop=(k == HID_K - 1))
            for m in range(KV_M):
                if m == 0:
                    last["act"] = nc.scalar.activation(sq_acc2, lat_ps[m],
                                                       AF.Square)
                else:
                    sq = tmps.tile([128, CH], F32R, tag="sq")
                    last["act"] = nc.scalar.activation(sq, lat_ps[m], AF.Square)
                    nc.gpsimd.tensor_tensor(sq_acc2, sq_acc2, sq, ALU.add)

            s_bc2 = rmsnorm_scale(sq_acc2, sca, KV_RANK)
            for m in range(KV_M):
                last["dve"] = nc.vector.tensor_mul(gsrc[:, m, :],
                                                   lat_ps[m], s_bc2)
            last["dve"] = rope(kr_ps, cosk_sb, sink_sb, gsrc[:, KV_M, :],
                               tmps, psX)

            # bounce + collective ride the gpsimd queue so the sync queue
            # (weight loads) never blocks behind them
            nc.sync.dma_start(out=bounceA, in_=gsrc[:, 0:KV_M + 1, :])
            nc.gpsimd.collective_compute(
                "AllGather", mybir.AluOpType.bypass, replica_groups=RG,
                ins=[bounceA.opt()], outs=[gathA.opt()])
            d_wkn = nc.sync.dma_start(out=wkn_sb, in_=w_kvb_n)
            d_wkv = nc.sync.dma_start(out=wkv_sb, in_=w_kvb_v)
            d_g = [d_wkn, d_wkv]
            for r in range(NCHUNK):
                rsl = slice(r * CH, (r + 1) * CH)
                d_g.append(nc.sync.dma_start(
                    out=lat_all[:, :, rsl], in_=gathA[r, :, 0:KV_M, :]))
                d_g.append(nc.sync.dma_start(
                    out=krot_sb[:, rsl], in_=gathA[r, :, KV_M, :]))

            # ---- q_a for own chunk (w_qa streamed per m-tile) ----
            sq_acc = tmps.tile([128, CH], F32R, tag="sqacc")
            for m in range(QA_M):
                wqa_m = wqa_pool.tile([128, HID_K, 128], BF16, tag="wqa")
                nc.scalar.dma_start(out=wqa_m, in_=w_qa[:, m, :, :])
                qa_ps = psA.tile([128, CH], F32, tag="mm")
                for k in range(HID_K):
                    last["pe"] = nc.tensor.matmul(
                        qa_ps, wqa_m[:, k, :],
                        ht_sb[:, k, :],
                        start=(k == 0), stop=(k == HID_K - 1))
                if m == 0:
                    last["act"] = nc.scalar.activation(sq_acc, qa_ps, AF.Square)
                else:
                    sq = tmps.tile([128, CH], F32R, tag="sq")
                    last["act"] = nc.scalar.activation(sq, qa_ps, AF.Square)
                    nc.gpsimd.tensor_tensor(sq_acc, sq_acc, sq, ALU.add)
                nc.scalar.copy(qa_sb[:, m, :], qa_ps)

            s_bc = rmsnorm_scale(sq_acc, sca, Q_RANK)
            for m in range(QA_M):
                last["dve"] = nc.vector.tensor_mul(gsrc[:, KV_M + 1 + m, :],
                                                   qa_sb[:, m, :], s_bc)

            # qa token-halves out + AllGathers (pipeline the CC stream)
            for hf in range(2):
                nc.sync.dma_start(
                    out=bounceB[hf],
                    in_=gsrc[:, KV_M + 1:, hf * HC:(hf + 1) * HC])
                nc.gpsimd.collective_compute(
                    "AllGather", mybir.AluOpType.bypass, replica_groups=RG,
                    ins=[bounceB[hf].opt()], outs=[gathB[hf].opt()])

        # ============ PHASE G: kn/v, q_b + attention by token-half ========
        pg = ExitStack()
        with pg:
            gconst = pg.enter_context(tc.tile_pool(name="gconst", bufs=1))
            cos_sb = gconst.tile([128, S], F32)
            d_cos = nc.scalar.dma_start(out=cos_sb, in_=cos2)
            sin_sb = gconst.tile([128, S], F32)
            d_sin = nc.scalar.dma_start(out=sin_sb, in_=sin2)
            wqb_sb = gconst.tile([128, QA_M, HPC * D_QK], BF16)
            d_wqb = nc.scalar.dma_start(out=wqb_sb, in_=w_qb)
            wo_sb = gconst.tile([128, HPC, HID], BF16)
            d_wo = nc.scalar.dma_start(out=wo_sb, in_=w_o)
            mask_sb = gconst.tile([128, 4, CH], F32)
            d_mk = nc.scalar.dma_start(out=mask_sb, in_=masks)

            qap = pg.enter_context(tc.tile_pool(name="qap", bufs=2))
            fence = pe_observe(d_cos, d_sin, d_wqb, d_wo,
                               d_mk, *d_g, last.get("pe"), last.get("act"),
                               last.get("dve"))

            tmps = pg.enter_context(tc.tile_pool(name="tmpsG", bufs=2))
            expp = pg.enter_context(tc.tile_pool(name="expp", bufs=4))
            attnp = pg.enter_context(tc.tile_pool(name="attnp", bufs=8))
            outp = pg.enter_context(tc.tile_pool(name="outp", bufs=3))
            scr = pg.enter_context(tc.tile_pool(name="scr", bufs=2))
            psB = pg.enter_context(tc.tile_pool(name="psB", bufs=2, space="PSUM"))
            psDn = pg.enter_context(tc.tile_pool(name="psDn", bufs=1, space="PSUM"))
            psSc = pg.enter_context(tc.tile_pool(name="psSc", bufs=2, space="PSUM"))
            psAt = pg.enter_context(tc.tile_pool(name="psAt", bufs=2, space="PSUM"))

            # k_nope FM per head (only needs gathA -> overlaps B1/B2)
            for h in range(HPC):
                for r in range(NCHUNK):
                    rsl = slice(r * CH, (r + 1) * CH)
                    ps = psB.tile([128, CH], F32, tag="mm")
                    for k in range(KV_M):
                        mm = nc.tensor.matmul(
                            ps, wkn_sb[:, k, h * 128:(h + 1) * 128],
                            lat_all[:, k, rsl],
                            start=(k == 0), stop=(k == KV_M - 1))
                        if k == 0 and fence is not None:
                            add_dep_helper(mm.ins, fence.ins, False,
                                           "order after phase fence")
                            fence = None
                    last["dve"] = nc.vector.tensor_copy(
                        out=kn_sb[:, h, rsl], in_=ps)

            # V token-major: latent tiles stationary
            for tt in range(KT):
                ps = psB.tile([128, HPC * D_V], F32, tag="mm")
                for k in range(KV_M):
                    nc.tensor.matmul(
                        ps, lat_all[:, k, tt * 128:(tt + 1) * 128],
                        wkv_sb[:, k, :],
                        start=(k == 0), stop=(k == KV_M - 1))
                last["dve"] = nc.vector.tensor_copy(out=v_sb[:, tt, :],
                                                    in_=ps)

            def emit_qb(hf, chunks=range(NCHUNK)):
                """q_b + rope for token-half hf of given chunks."""
                for r in chunks:
                    hsl = slice(r * CH + hf * HC, r * CH + (hf + 1) * HC)
                    qa_r = qap.tile([128, QA_M, HC], BF16, tag="qar")
                    nc.sync.dma_start(out=qa_r, in_=gathB[hf][r])
                    for m in range(QB_M):
                        qb_ps = psB.tile([128, HC], F32, tag="mm")
                        for k in range(QA_M):
                            nc.tensor.matmul(
                                qb_ps, wqb_sb[:, k, m * 128:(m + 1) * 128],
                                qa_r[:, k, :],
                                start=(k == 0), stop=(k == QA_M - 1))
                        if m < HPC:  # nope part: cast copy
                            last["act"] = nc.scalar.copy(
                                qpass_sb[:, m, hsl], qb_ps)
                        else:        # rot part
                            last["dve"] = rope(qb_ps, cos_sb[:, hsl],
                                               sin_sb[:, hsl],
                                               qrot_sb[:, m - HPC, hsl],
                                               tmps, psSc, width=HC)

            def emit_attn(qc):
                """attention + o-proj for one 512-token chunk."""
                if True:
                    csl = slice(qc * CH, (qc + 1) * CH)
                    nkt = 4 * qc + 4
                    attn_tiles = []
                    for h in range(HPC):
                        j, par = h // 2, h % 2
                        poff = 64 * par
                        at_ps = psAt.tile([128, CH], F32, tag="attn")
                        dn_ps = psDn.tile([1, CH], F32, tag="denom")
                        for kt in range(nkt):
                            ksl = slice(kt * 128, (kt + 1) * 128)
                            sc_ps = psSc.tile([128, CH], F32, tag="scores")
                            nc.tensor.matmul(
                                sc_ps, kn_sb[:, h, ksl], qpass_sb[:, h, csl],
                                start=True, stop=False)
                            nc.tensor.matmul(
                                sc_ps,
                                krot_sb[poff:poff + 64, ksl],
                                qrot_sb[poff:poff + 64, j, csl],
                                start=False, stop=True)
                            if kt >= 4 * qc:  # diagonal block: causal mask
                                nc.vector.tensor_tensor(
                                    sc_ps, sc_ps, mask_sb[:, kt - 4 * qc, :],
                                    ALU.add)
                            ex = expp.tile([128, CH], BF16, tag="exp")
                            nc.scalar.activation(ex, sc_ps, AF.Exp,
                                                 scale=SCALING)
                            nc.tensor.matmul(
                                at_ps, v_sb[:, kt, h * 128:(h + 1) * 128], ex,
                                start=(kt == 0), stop=(kt == nkt - 1))
                            nc.tensor.matmul(
                                dn_ps, ones_col_bf, ex,
                                start=(kt == 0), stop=(kt == nkt - 1))
                        # normalize: attn *= 1/denom
                        rc = scr.tile([1, CH], F32, tag="recip")
                        nc.vector.reciprocal(rc, dn_ps)
                        rb_sb = scr.tile([128, CH], F32, tag="rbs")
                        nc.gpsimd.partition_broadcast(rb_sb, rc, 128)
                        at_sb = attnp.tile([128, CH], BF16, tag="attn")
                        nc.vector.tensor_mul(at_sb, at_ps, rb_sb)
                        attn_tiles.append(at_sb)

                    # o-proj for this chunk
                    for tt in range(CH // 128):
                        for hck in range(NCHUNK):
                            o_ps = psB.tile([128, CH], F32, tag="mm")
                            for h in range(HPC):
                                nc.tensor.matmul(
                                    o_ps,
                                    attn_tiles[h][:, tt * 128:(tt + 1) * 128],
                                    wo_sb[:, h, hck * CH:(hck + 1) * CH],
                                    start=(h == 0), stop=(h == HPC - 1))
                            o_sb = outp.tile([128, CH], BF16, tag="osb")
                            nc.scalar.copy(o_sb, o_ps)
                            r0 = qc * CH + tt * 128
                            nc.sync.dma_start(
                                out=out[r0:r0 + 128, hck * CH:(hck + 1) * CH],
                                in_=o_sb)

            emit_qb(0)
            for r in range(NCHUNK):
                emit_qb(1, chunks=[r])
                emit_attn(r)


# ---------------------------------------------------------------------------
# host side
# ---------------------------------------------------------------------------

_ILV = np.concatenate([np.arange(0, 64, 2), np.arange(1, 64, 2)])  # interleave


def _rot_half_mat():
    r = np.zeros((64, 64), np.float32)
    for m in range(32):
        r[m + 32, m] = -1.0
    for m in range(32, 64):
        r[m - 32, m] = 1.0
    r2 = np.zeros((128, 128), np.float32)
    r2[:64, :64] = r
    r2[64:, 64:] = r
    return r2


def _masks():
    mk = np.zeros((4, 128, CH), np.float32)
    i = np.arange(128)[:, None]
    j = np.arange(CH)[None, :]
    for v in range(4):
        mk[v] = np.where(v * 128 + i <= j, 0.0, NEG)
    return mk


def make_in_maps(hidden_states, w_qa, g_qa, w_qb, w_kva, g_kva, w_kvb, w_o,
                 cos, sin):
    bf = ml_dtypes.bfloat16
    w_qb_eff = (w_qb * g_qa[:, None]).astype(np.float32)
    w_kvb_eff = (w_kvb * g_kva[:, None]).astype(np.float32)
    r2 = _rot_half_mat()
    mk = np.ascontiguousarray(_masks().transpose(1, 0, 2))
    wqa_t = np.ascontiguousarray(
        w_qa.reshape(HID_K, 128, QA_M, 128).transpose(1, 2, 0, 3)).astype(bf)

    in_maps = []
    for c in range(8):
        b, g = c // 4, c % 4
        heads = range(4 * g, 4 * g + 4)
        tsl = slice(g * CH, (g + 1) * CH)   # own token chunk

        hT = np.ascontiguousarray(
            hidden_states[b, tsl].T.reshape(HID_K, 128, CH)
            .transpose(1, 0, 2)).astype(bf)

        # w_qb columns for these heads: 4 nope blocks then 2 rot tiles
        nope_cols = np.concatenate(
            [w_qb_eff[:, h * D_QK: h * D_QK + D_NOPE] for h in heads], axis=1)
        rot_cols = np.concatenate(
            [w_qb_eff[:, h * D_QK + D_NOPE: (h + 1) * D_QK][:, _ILV]
             for h in heads], axis=1)
        wqb_c = np.ascontiguousarray(
            np.concatenate([nope_cols, rot_cols], axis=1)
            .reshape(QA_M, 128, HPC * D_QK).transpose(1, 0, 2)).astype(bf)

        # w_kva: latent cols + interleaved rot cols duplicated
        rotw = w_kva[:, KV_RANK:KV_RANK + D_ROPE][:, _ILV]
        wkva_c = np.ascontiguousarray(
            np.concatenate([w_kva[:, :KV_RANK], rotw, rotw], axis=1)
            .reshape(HID_K, 128, KV_RANK + 128).transpose(1, 0, 2)).astype(bf)

        wkn_c = np.ascontiguousarray(
            np.concatenate(
                [w_kvb_eff[:, h * (D_NOPE + D_V): h * (D_NOPE + D_V) + D_NOPE]
                 for h in heads], axis=1)
            .reshape(KV_M, 128, HPC * D_NOPE).transpose(1, 0, 2)).astype(bf)
        wkv_c = np.ascontiguousarray(
            np.concatenate(
                [w_kvb_eff[:, h * (D_NOPE + D_V) + D_NOPE:
                           (h + 1) * (D_NOPE + D_V)]
                 for h in heads], axis=1)
            .reshape(KV_M, 128, HPC * D_V).transpose(1, 0, 2)).astype(bf)

        wo_c = np.ascontiguousarray(
            np.stack([w_o[h * D_V:(h + 1) * D_V, :] for h in heads], axis=1)
        ).astype(bf)

        cosT = np.ascontiguousarray(cos[b].T).astype(np.float32)  # [64, S]
        sinT = np.ascontiguousarray(sin[b].T).astype(np.float32)
        cos2 = np.concatenate([cosT, cosT], axis=0)
        sin2 = np.concatenate([sinT, sinT], axis=0)

        in_maps.append({
            "hT": hT,
            "w_qa": wqa_t,
            "w_qb": wqb_c,
            "w_kva": wkva_c,
            "w_kvb_n": wkn_c,
            "w_kvb_v": wkv_c,
            "w_o": wo_c,
            "cos2": cos2,
            "sin2": sin2,
            "cosk": np.ascontiguousarray(cos2[:, tsl]),
            "sink": np.ascontiguousarray(sin2[:, tsl]),
            "r2": r2,
            "masks": mk,
        })
    return in_maps


_NC_CACHE = {}


def get_nc():
    if "nc" not in _NC_CACHE:
        _NC_CACHE["nc"] = build_nc()
    return _NC_CACHE["nc"]


def run(in_maps, **kw):
    nc = get_nc()
    return run_bass_kernel_spmd(nc, in_maps, list(range(8)), **kw)


def kernel(hidden_states, w_qa, g_qa, w_qb, w_kva, g_kva, w_kvb, w_o, cos, sin):
    args = [np.asarray(a) for a in (hidden_states, w_qa, g_qa, w_qb, w_kva,
                                    g_kva, w_kvb, w_o, cos, sin)]
    in_maps = make_in_maps(*args)
    res = run(in_maps).results
    out = np.zeros((B, S, HID), np.float32)
    for c in range(8):
        out[c // 4] += res[c]["out"].astype(np.float32)
    return out
